# revision 9
# baseline (speedup 1.0000x reference)
"""HGConv kernel for Trainium2: 8-way data-parallel over batch.

Math (per batch b, transposed [d, e] layout so softmaxes reduce the free axis):
    aggT[d,e]  = sum_m nf[m,d] * inc[m,e]            (the ONLY big matmul)
    scoresT    = W_att @ aggT
    attnT      = softmax_e(scoresT)
    mulT       = aggT * attnT
    a[e]       = w_eff @ mulT          # w_eff = ec_att_w @ W_proj (host-folded)
    w          = softmax_e(a)
    q[d]       = sum_e mulT[d,e] * w[e]
    logits     = W3 @ q + b2           # W3 = fc_w @ ec_proj_w @ W_proj (host-folded)
  (pooled = sum_e (W_proj@mulT)*w = W_proj @ (mulT @ w) -- so the [d,e]-sized
   edge_feat tensor is never materialized; the e-reduction happens on mulT.)

Engineering notes:
  - inc is 0/1 -> host-cast to fp8_e4m3 (EXACT), quartering the dominant
    HBM stream (16.8 MB -> 4.2 MB/core); nf host-cast to bf16.
  - single bf16(nf) x fp8(inc) matmul per m-chunk half, fp32 PSUM accum;
    no on-device casts in the main loop at all.
  - both operands packed on host as [128, chunk-major free] so every DMA
    line is >=1 KB contiguous; inc streams on the sync HWDGE ring, nf and
    weights ride the gpsimd SWDGE ring.
  - w_eff enters as a [128,128] column-replicated stationary so a[e] is
    computed already broadcast across partitions (no [1,E] row ops).
  - tail elementwise in bf16 (2x DVE), tail matmul moving operands bf16
    (1 cycle/row vs 4 for fp32); exp skips max-subtraction (|scores|<=~45,
    f32-safe; checked on the input distribution).
"""

import sys

import numpy as np

sys.path.insert(0, "/opt/trn_rl_repo")

B, M, E, D, NCAT = 8, 4096, 1024, 128, 64
P = 128
NCHUNK = M // P                      # 32 m-chunks of 128
GROUPS = [2, 2, 4, 4, 4, 4, 4, 4, 4]  # m-chunks per DMA group (small first)
assert sum(GROUPS) == NCHUNK
EH = 512                             # PSUM bank width in fp32

_cache = {}


def _build_nc():
    import concourse.bacc as bacc
    import concourse.bass as bass
    import concourse.mybir as mybir
    from concourse.tile import TileContext

    f32 = mybir.dt.float32
    bf16 = mybir.dt.bfloat16
    fp8 = mybir.dt.float8e4
    AF = mybir.ActivationFunctionType
    ALU = mybir.AluOpType

    nc = bacc.Bacc(None)

    # host-packed operands: partition-major, chunk-major free axis
    inc_p = nc.dram_tensor("inc_p", [P, NCHUNK * E], fp8, kind="ExternalInput")
    nf_p = nc.dram_tensor("nf_p", [P, NCHUNK * D], bf16, kind="ExternalInput")
    # wpack cols: w_attT(128) | w_eff_rep(128) | w3T(64)
    wpack = nc.dram_tensor("wpack", [P, 320], bf16, kind="ExternalInput")
    b2 = nc.dram_tensor("b2_col", [NCAT, 1], f32, kind="ExternalInput")
    out_d = nc.dram_tensor("logits", [NCAT, 1], f32, kind="ExternalOutput")
    warm_d = nc.dram_tensor("warm_sink", [1, 1], f32, kind="ExternalOutput")

    with TileContext(nc) as tc:
        with (
            tc.tile_pool(name="const", bufs=1) as cpool,
            tc.tile_pool(name="work", bufs=1) as work,
            tc.tile_pool(name="psb", bufs=2, space=bass.MemorySpace.PSUM) as psb,
            tc.tile_pool(name="pss", bufs=1, space=bass.MemorySpace.PSUM) as pss,
        ):
            inc_sb = cpool.tile([P, NCHUNK * E], fp8)
            nf_sb = cpool.tile([P, NCHUNK * D], bf16)
            wpack_sb = cpool.tile([P, 320], bf16)
            b2_sb = cpool.tile([NCAT, 1], f32)

            # PE warm-up: the HAM clock gate needs ~3.4us of sustained PE
            # activity to lift the 1.2 -> 2.4 GHz throttle.  Burn a few dummy
            # matmuls on a zeroed tile while the first DMAs are in flight
            # so the PE is busy (and warming) when the first chunk lands.
            warm_sb = cpool.tile([P, 512], bf16)
            nc.vector.memset(warm_sb[:], 0.0)
            warm_ps = pss.tile([P, 512], f32, tag="warm")
            NWARM = 4
            for i in range(NWARM):
                nc.tensor.matmul(
                    warm_ps[:], warm_sb[:, 0:128], warm_sb[:],
                    start=True, stop=True,
                )

            # stream inc groups alternating across BOTH HWDGE rings (sync +
            # scalar) so descriptor generation and completion semaphores run
            # in parallel; nf + weights ride the gpsimd SWDGE ring.  nf
            # group 0 goes on the scalar ring up front so chunk 0 is ready
            # as soon as inc group 0 lands.
            edges = np.cumsum([0] + GROUPS)
            for g, (c0, c1) in enumerate(zip(edges[:-1], edges[1:])):
                if g == 0:
                    nc.scalar.dma_start(
                        nf_sb[:, c0 * D:c1 * D], nf_p[:, c0 * D:c1 * D]
                    )
                else:
                    nc.gpsimd.dma_start(
                        nf_sb[:, c0 * D:c1 * D], nf_p[:, c0 * D:c1 * D]
                    )
                ring = nc.sync if g % 2 == 0 else nc.scalar
                ring.dma_start(
                    inc_sb[:, c0 * E:c1 * E], inc_p[:, c0 * E:c1 * E]
                )
                if g == 1:
                    nc.gpsimd.dma_start(wpack_sb[:], wpack[:])
                    nc.gpsimd.dma_start(b2_sb[:], b2[:])

            # ---- aggT[d,e] accumulation over 32 m-chunks ----
            agg_ps = psb.tile([P, E], f32, tag="big")
            for n in range(NCHUNK):
                lhs = nf_sb[:, n * D:(n + 1) * D]
                first, last = n == 0, n == NCHUNK - 1
                nc.tensor.matmul(
                    agg_ps[:, 0:EH], lhs, inc_sb[:, n * E:n * E + EH],
                    start=first, stop=last,
                )
                nc.tensor.matmul(
                    agg_ps[:, EH:E], lhs, inc_sb[:, n * E + EH:(n + 1) * E],
                    start=first, stop=last,
                )

            w_attT_sb = wpack_sb[:, 0:128]
            weffr_sb = wpack_sb[:, 128:256]
            w3T_sb = wpack_sb[:, 256:320]

            # ---- tail, pipelined in E-halves ----
            # t = exp(scores) * agg stays UN-normalized; the attn 1/rowsum
            # folds into the ab-matmul stationary (w2 = w_eff * rinv) and the
            # q reduction (one STT with rinv as the scalar), so no separate
            # normalize pass is needed.
            agg_sb = work.tile([P, E], bf16)
            scr_ps = psb.tile([P, E], f32, tag="big")
            exp_sb = work.tile([P, E], bf16)
            rsum0 = work.tile([P, 1], f32)
            rsum1 = work.tile([P, 1], f32)
            # h1 via ACT, h0 via DVE so both copies overlap; scr/exp chase
            # each half as it lands.
            nc.scalar.copy(agg_sb[:, EH:E], agg_ps[:, EH:E])
            nc.vector.tensor_copy(agg_sb[:, 0:EH], agg_ps[:, 0:EH])
            nc.tensor.matmul(scr_ps[:, EH:E], w_attT_sb, agg_sb[:, EH:E],
                             start=True, stop=True)
            nc.tensor.matmul(scr_ps[:, 0:EH], w_attT_sb, agg_sb[:, 0:EH],
                             start=True, stop=True)
            nc.scalar.activation(exp_sb[:, EH:E], scr_ps[:, EH:E], AF.Exp,
                                 bias=0.0, accum_out=rsum1[:])
            nc.scalar.activation(exp_sb[:, 0:EH], scr_ps[:, 0:EH], AF.Exp,
                                 bias=0.0, accum_out=rsum0[:])
            t_sb = work.tile([P, E], bf16)
            nc.vector.tensor_mul(t_sb[:, EH:E], exp_sb[:, EH:E],
                                 agg_sb[:, EH:E])
            nc.vector.tensor_mul(t_sb[:, 0:EH], exp_sb[:, 0:EH],
                                 agg_sb[:, 0:EH])
            rsum = work.tile([P, 1], f32)
            nc.vector.tensor_add(rsum[:], rsum0[:], rsum1[:])
            rinv = work.tile([P, 1], f32)
            nc.vector.reciprocal(rinv[:], rsum[:])
            w2_sb = work.tile([P, P], bf16)
            nc.vector.tensor_scalar_mul(w2_sb[:], weffr_sb, rinv[:])

            # ---- a (row-replicated) = (w_eff*rinv) @ t ; softmax over e ----
            ab_ps = psb.tile([P, E], f32, tag="big")
            expb = work.tile([P, E], bf16)
            asum0 = work.tile([P, 1], f32)
            asum1 = work.tile([P, 1], f32)
            prod = work.tile([P, E], bf16)
            q0 = work.tile([P, 1], f32)
            q1 = work.tile([P, 1], f32)
            nc.tensor.matmul(ab_ps[:, 0:EH], w2_sb[:], t_sb[:, 0:EH],
                             start=True, stop=True)
            nc.tensor.matmul(ab_ps[:, EH:E], w2_sb[:], t_sb[:, EH:E],
                             start=True, stop=True)
            nc.scalar.activation(expb[:, 0:EH], ab_ps[:, 0:EH], AF.Exp,
                                 bias=0.0, accum_out=asum0[:])
            nc.scalar.activation(expb[:, EH:E], ab_ps[:, EH:E], AF.Exp,
                                 bias=0.0, accum_out=asum1[:])
            # q_h = sum_e t * rinv * expb  (rinv re-applies attn normalize)
            nc.vector.scalar_tensor_tensor(
                prod[:, 0:EH], t_sb[:, 0:EH], rinv[:], expb[:, 0:EH],
                op0=ALU.mult, op1=ALU.mult, accum_out=q0[:],
            )
            nc.vector.scalar_tensor_tensor(
                prod[:, EH:E], t_sb[:, EH:E], rinv[:], expb[:, EH:E],
                op0=ALU.mult, op1=ALU.mult, accum_out=q1[:],
            )
            asum = work.tile([P, 1], f32)
            nc.vector.tensor_add(asum[:], asum0[:], asum1[:])
            ainv = work.tile([P, 1], f32)
            nc.vector.reciprocal(ainv[:], asum[:])
            q_raw = work.tile([P, 1], f32)
            nc.vector.tensor_add(q_raw[:], q0[:], q1[:])
            q_sb = work.tile([P, 1], bf16)
            nc.vector.tensor_scalar_mul(q_sb[:], q_raw[:], ainv[:])
            log_ps = pss.tile([NCAT, 1], f32, tag="tiny")
            nc.tensor.matmul(log_ps[:], w3T_sb, q_sb[:], start=True, stop=True)
            logit_sb = work.tile([NCAT, 1], f32)
            nc.vector.tensor_add(logit_sb[:], log_ps[:], b2_sb[:])
            nc.sync.dma_start(out_d[:], logit_sb[:])
            # sink for the warm-up PSUM so the release pass sees a reader
            warm_red = work.tile([1, 1], f32)
            nc.vector.tensor_copy(warm_red[:], warm_ps[0:1, 0:1])
            nc.gpsimd.dma_start(warm_d[:], warm_red[:])

    nc.finalize()
    return nc


def _get_nc():
    if "nc" not in _cache:
        _cache["nc"] = _build_nc()
    return _cache["nc"]


def kernel(node_feats, inc_mat, W_att, W_proj, ec_att_w, ec_proj_w, ec_proj_b,
           fc_w, fc_b, **trace_kw):
    import ml_dtypes
    from concourse.bass_utils import run_bass_kernel_spmd

    node_feats = np.asarray(node_feats, dtype=np.float32)
    inc_mat = np.asarray(inc_mat, dtype=np.float32)
    W_att = np.asarray(W_att, np.float32)
    W_proj = np.asarray(W_proj, np.float32)
    ec_att_w = np.asarray(ec_att_w, np.float32)
    ec_proj_w = np.asarray(ec_proj_w, np.float32)
    ec_proj_b = np.asarray(ec_proj_b, np.float32)
    fc_w = np.asarray(fc_w, np.float32)
    fc_b = np.asarray(fc_b, np.float32)

    # host-folded weights (constant preprocessing, O(D^2) flops)
    w_eff = (ec_att_w @ W_proj).ravel()                       # [D]
    W3 = fc_w @ ec_proj_w @ W_proj                            # [NCAT, D]
    b2 = (fc_w @ ec_proj_b + fc_b).reshape(NCAT, 1)           # [NCAT, 1]
    wpack = np.concatenate(
        [
            np.ascontiguousarray(W_att.T),                    # [D, D]
            np.tile(w_eff[:, None], (1, D)),                  # [D, D] replicated
            np.ascontiguousarray(W3.T),                       # [D, NCAT]
        ],
        axis=1,
    ).astype(ml_dtypes.bfloat16)

    # pack per-core operands: "(n p) x -> p (n x)" so DMA lines are contiguous
    nf_pack = (
        node_feats.reshape(B, NCHUNK, P, D).transpose(0, 2, 1, 3)
        .reshape(B, P, NCHUNK * D).astype(ml_dtypes.bfloat16)
    )
    inc_pack = (
        inc_mat.reshape(B, NCHUNK, P, E).transpose(0, 2, 1, 3)
        .reshape(B, P, NCHUNK * E).astype(ml_dtypes.float8_e4m3)
    )

    shared = {"wpack": wpack, "b2_col": np.ascontiguousarray(b2)}
    in_maps = [
        {"nf_p": nf_pack[b], "inc_p": inc_pack[b], **shared}
        for b in range(B)
    ]
    res = run_bass_kernel_spmd(_get_nc(), in_maps, list(range(B)), **trace_kw)
    out = np.stack([res.results[b]["logits"].reshape(NCAT) for b in range(B)])
    if trace_kw:
        return out, res
    return out
